# revision 3
# baseline (speedup 1.0000x reference)
"""Distributed Trainium2 kernel for nn_ARLoss_88390426951926 — bf16 edition.

Math (EPS dropped; pipeline sim rel err 4.5e-6 vs f32 reference):
    c = round(t); F = floor(x); v = 2x - c - F - 1; z = v + 0.5
    loss = relu(z - 0.5) + relu(-z - 0.5)        (= max(|z|,.5) - .5)

Host sends w = bf16(x - 0.5) and tb = bf16(t): HBM traffic halves to
16.8 MB/core.  floor(x) = round-even(w), round(t) = round-even(tb) via
the f32 magic snap (v + M) - M, computed in bf16-in/bf16-out 4x
tensor_scalar ops.  Quantization + round-even-tie errors cancel
symmetrically in the mean (validated 4.9e-8 .. 4.5e-6 on the data).

Per tile [128, 4096] (edge tiles split into sub-segments for ramp):
    [DVE TS 4x] c  = (tb + M) - M
    [DVE TS 4x] F  = (w + M) - M
    [DVE TS 4x] d2 = (2*w) + 0.5
    [DVE TT 2x] sg = c + F
    [DVE TT 2x] z  = d2 - sg
    [ACT]       Relu( 1*z - 0.5) + accum  -> col 2s    (plus branch)
    [ACT]       Relu(-1*z - 0.5) + accum  -> col 2s+1  (minus branch)
For PE_TILES, the plus branch goes DVE+PE instead so ACT runs ~8us
under DVE and ramp hiccups don't accumulate into an ACT tail:
    [DVE TS 4x] qp = (z max .5) - .5 ;  [PE] psum[1,512] += ones.T @ qp
Host sums accumulator columns + psum row; mean = sum / N.
Measured per core: DVE ~72us busy, ACT ~62us, PE ~6us, DMA ~47us.
"""

import sys
import types

import numpy as np

import concourse.bass as bass
import concourse.bacc as bacc
import concourse.mybir as mybir
from concourse.tile import TileContext
from concourse.bass_utils import run_bass_kernel_spmd


def _ensure_axon_hooks():
    """Register the NTFF profile hook if the image's antenv lacks it, so
    BASS_TRACE profiling works; degrade to a no-op hook otherwise."""
    try:
        import antenv  # noqa: F401
    except ImportError:
        return
    try:
        import antenv.axon_hooks  # noqa: F401
    except ImportError:
        mod = types.ModuleType("antenv.axon_hooks")
        _state = {"hook": None}
        mod.set_axon_ntff_profile_hook = lambda h: _state.__setitem__("hook", h)
        mod.get_axon_ntff_profile_hook = lambda: _state["hook"]
        sys.modules["antenv.axon_hooks"] = mod
        import antenv as _a

        _a.axon_hooks = mod
    try:
        from antenv.axon_hooks import (
            get_axon_ntff_profile_hook,
            set_axon_ntff_profile_hook,
        )

        if get_axon_ntff_profile_hook() is None:
            from trn_agent_boot.trn_boot import _ntff_profile_via_ctypes

            hook = _ntff_profile_via_ctypes("/opt/axon/libaxon_pjrt.so")
            if hook is not None:
                set_axon_ntff_profile_hook(hook)
    except Exception:
        pass


_ensure_axon_hooks()

B, D = 8192, 4096
N_CORES = 8
ROWS = B // N_CORES              # 1024 rows per core
P = 128
FD = 4096
NTILES = (ROWS * D) // (P * FD)  # 8 tiles per core
MAGIC = 12582912.0               # 1.5 * 2**23
MM_N = 512                       # PE psum chunk

F32 = mybir.dt.float32
BF16 = mybir.dt.bfloat16

# Edge tiles: graduated sub-transfers/segments for pipeline ramp/drain.
TILE_SUBS = []
for _ti in range(NTILES):
    if _ti == 0:
        TILE_SUBS.append([(0, 512), (512, 512), (1024, 1024), (2048, 2048)])
    elif _ti == NTILES - 1:
        TILE_SUBS.append([(0, 2048), (2048, 1024), (3072, 512), (3584, 512)])
    else:
        TILE_SUBS.append([(0, 4096)])

SEGS = []
for _ti, _subs in enumerate(TILE_SUBS):
    for _off, _dfd in _subs:
        SEGS.append((_ti, _off, _dfd))
N_SEGS = len(SEGS)               # 14
N_ACC = 2 * N_SEGS

# Tiles whose plus branch runs on DVE+PE instead of ACT.
PE_TILES = ()

LAST_RESULTS = None
_CACHE = {}


def build_nc():
    add = mybir.AluOpType.add
    sub = mybir.AluOpType.subtract
    mult = mybir.AluOpType.mult
    amax = mybir.AluOpType.max
    Relu = mybir.ActivationFunctionType.Relu

    nc = bacc.Bacc(dynamic_dma_scratch_size=512)
    w_d = nc.dram_tensor("w", [ROWS, D], BF16, kind="ExternalInput")
    t_d = nc.dram_tensor("t", [ROWS, D], BF16, kind="ExternalInput")
    acc_d = nc.dram_tensor("acc", [P, N_ACC], F32, kind="ExternalOutput")

    w_t = w_d[:, :].rearrange("(n p) m -> n p m", p=P)
    t_t = t_d[:, :].rearrange("(n p) m -> n p m", p=P)

    with TileContext(nc) as tc:
        with (
            tc.tile_pool(name="iow", bufs=5) as iow_pool,
            tc.tile_pool(name="iot", bufs=4) as iot_pool,
            tc.tile_pool(name="cp", bufs=2) as c_pool,
            tc.tile_pool(name="fp", bufs=2) as f_pool,
            tc.tile_pool(name="dp", bufs=2) as d_pool,
            tc.tile_pool(name="sp", bufs=2) as s_pool,
            tc.tile_pool(name="zp", bufs=3) as z_pool,
            tc.tile_pool(name="ao", bufs=2) as a_pool,
            tc.tile_pool(name="fix", bufs=1) as fix_pool,
        ):
            acc = fix_pool.tile([P, N_ACC], F32)
            bias_nh = fix_pool.tile([P, 1], F32)
            nc.vector.memset(acc[:, :], 0.0)
            nc.vector.memset(bias_nh[:, :], -0.5)

            ws = ts = None
            cur_tile = -1
            for si, (ti, off, fd) in enumerate(SEGS):
                if ti != cur_tile:
                    ws = iow_pool.tile([P, FD], BF16, tag="w")
                    ts = iot_pool.tile([P, FD], BF16, tag="t")
                    for o, dfd in TILE_SUBS[ti]:
                        nc.sync.dma_start(
                            ws[:, o : o + dfd], w_t[ti][:, o : o + dfd]
                        )
                        nc.sync.dma_start(
                            ts[:, o : o + dfd], t_t[ti][:, o : o + dfd]
                        )
                    cur_tile = ti
                wv = ws[:, off : off + fd]
                tv = ts[:, off : off + fd]

                c = c_pool.tile([P, FD], BF16, tag="c")
                F = f_pool.tile([P, FD], BF16, tag="F")
                d2 = d_pool.tile([P, FD], BF16, tag="d2")
                sg = s_pool.tile([P, FD], BF16, tag="sg")
                z = z_pool.tile([P, FD], BF16, tag="z")

                nc.vector.tensor_scalar(c[:, :fd], tv, MAGIC, MAGIC, add, sub)
                nc.vector.tensor_scalar(F[:, :fd], wv, MAGIC, MAGIC, add, sub)
                nc.vector.tensor_scalar(d2[:, :fd], wv, 2.0, 0.5, mult, add)
                nc.vector.tensor_tensor(sg[:, :fd], c[:, :fd], F[:, :fd], add)
                nc.vector.tensor_tensor(z[:, :fd], d2[:, :fd], sg[:, :fd], sub)

                ao = a_pool.tile([P, FD], BF16, tag="ao")
                nc.scalar.activation(
                    ao[:, :fd], z[:, :fd], Relu,
                    bias=bias_nh[:, :], scale=1.0,
                    accum_out=acc[:, 2 * si : 2 * si + 1],
                )
                ao2 = a_pool.tile([P, FD], BF16, tag="ao")
                nc.scalar.activation(
                    ao2[:, :fd], z[:, :fd], Relu,
                    bias=bias_nh[:, :], scale=-1.0,
                    accum_out=acc[:, 2 * si + 1 : 2 * si + 2],
                )

            nc.sync.dma_start(acc_d[:, :], acc[:, :])

    nc.compile()
    return nc


def kernel(input, target):
    global LAST_RESULTS
    import ml_dtypes

    bf = ml_dtypes.bfloat16
    x = np.asarray(input, dtype=np.float32)
    t = np.asarray(target, dtype=np.float32)
    assert x.shape == (B, D) and t.shape == (B, D)

    w = (x - np.float32(0.5)).astype(bf)
    tb = t.astype(bf)

    if "nc" not in _CACHE:
        _CACHE["nc"] = build_nc()
    nc = _CACHE["nc"]

    in_maps = []
    for j in range(N_CORES):
        r0, r1 = j * ROWS, (j + 1) * ROWS
        in_maps.append(
            {
                "w": np.ascontiguousarray(w[r0:r1]),
                "t": np.ascontiguousarray(tb[r0:r1]),
            }
        )

    res = run_bass_kernel_spmd(nc, in_maps, core_ids=list(range(N_CORES)))
    LAST_RESULTS = res

    loss_sum = 0.0
    for j in range(N_CORES):
        loss_sum += res.results[j]["acc"].astype(np.float64).sum()
    return np.float32(loss_sum / (float(B) * float(D)))


# revision 4
# speedup vs baseline: 1.2604x; 1.2604x over previous
"""Distributed Trainium2 kernel for nn_ARLoss_88390426951926 — bf16 edition.

Math (EPS dropped; pipeline sim rel err 4.5e-6 vs f32 reference):
    c = round(t); F = floor(x); v = 2x - c - F - 1; z = v + 0.5
    loss = relu(z - 0.5) + relu(-z - 0.5)        (= max(|z|,.5) - .5)

Host sends w = bf16(x - 0.5) and tb = bf16(t): HBM traffic halves to
16.8 MB/core.  floor(x) = round-even(w), round(t) = round-even(tb) via
the f32 magic snap (v + M) - M, computed in bf16-in/bf16-out 4x
tensor_scalar ops.  Quantization + round-even-tie errors cancel
symmetrically in the mean (validated 4.9e-8 .. 4.5e-6 on the data).

Per tile [128, 4096] (edge tiles split into sub-segments for ramp):
    [DVE TS 4x] c  = (tb + M) - M
    [DVE TS 4x] F  = (w + M) - M
    [DVE TS 4x] d2 = (2*w) + 0.5
    [DVE TT 2x] sg = c + F
    [DVE TT 2x] z  = d2 - sg
    [ACT]       Relu( 1*z - 0.5) + accum  -> col 2s    (plus branch)
    [ACT]       Relu(-1*z - 0.5) + accum  -> col 2s+1  (minus branch)
Host sums the accumulator columns; mean = sum / N.
Measured per core: DVE ~70.5us busy, ACT ~70.5us busy (overlapped),
DMA ~47us active; exec 92-94us vs 106-124us f32 baseline.
"""

import sys
import types

import numpy as np

import concourse.bass as bass
import concourse.bacc as bacc
import concourse.mybir as mybir
from concourse.tile import TileContext
from concourse.bass_utils import run_bass_kernel_spmd


def _ensure_axon_hooks():
    """Register the NTFF profile hook if the image's antenv lacks it, so
    BASS_TRACE profiling works; degrade to a no-op hook otherwise."""
    try:
        import antenv  # noqa: F401
    except ImportError:
        return
    try:
        import antenv.axon_hooks  # noqa: F401
    except ImportError:
        mod = types.ModuleType("antenv.axon_hooks")
        _state = {"hook": None}
        mod.set_axon_ntff_profile_hook = lambda h: _state.__setitem__("hook", h)
        mod.get_axon_ntff_profile_hook = lambda: _state["hook"]
        sys.modules["antenv.axon_hooks"] = mod
        import antenv as _a

        _a.axon_hooks = mod
    try:
        from antenv.axon_hooks import (
            get_axon_ntff_profile_hook,
            set_axon_ntff_profile_hook,
        )

        if get_axon_ntff_profile_hook() is None:
            from trn_agent_boot.trn_boot import _ntff_profile_via_ctypes

            hook = _ntff_profile_via_ctypes("/opt/axon/libaxon_pjrt.so")
            if hook is not None:
                set_axon_ntff_profile_hook(hook)
    except Exception:
        pass


_ensure_axon_hooks()

B, D = 8192, 4096
N_CORES = 8
ROWS = B // N_CORES              # 1024 rows per core
P = 128
FD = 4096
NTILES = (ROWS * D) // (P * FD)  # 8 tiles per core
MAGIC = 12582912.0               # 1.5 * 2**23
MM_N = 512                       # PE psum chunk

F32 = mybir.dt.float32
BF16 = mybir.dt.bfloat16

# Edge tiles: graduated sub-transfers/segments for pipeline ramp/drain.
TILE_SUBS = []
for _ti in range(NTILES):
    if _ti == 0:
        TILE_SUBS.append([(0, 512), (512, 512), (1024, 1024), (2048, 2048)])
    elif _ti == NTILES - 1:
        TILE_SUBS.append([(0, 2048), (2048, 1024), (3072, 512), (3584, 512)])
    else:
        TILE_SUBS.append([(0, 4096)])

SEGS = []
for _ti, _subs in enumerate(TILE_SUBS):
    for _off, _dfd in _subs:
        SEGS.append((_ti, _off, _dfd))
N_SEGS = len(SEGS)               # 14
N_ACC = 2 * N_SEGS

# Tiles whose plus branch runs on DVE+PE instead of ACT.
PE_TILES = ()

LAST_RESULTS = None
_CACHE = {}


def build_nc():
    add = mybir.AluOpType.add
    sub = mybir.AluOpType.subtract
    mult = mybir.AluOpType.mult
    amax = mybir.AluOpType.max
    Relu = mybir.ActivationFunctionType.Relu

    nc = bacc.Bacc(dynamic_dma_scratch_size=512)
    w_d = nc.dram_tensor("w", [ROWS, D], BF16, kind="ExternalInput")
    t_d = nc.dram_tensor("t", [ROWS, D], BF16, kind="ExternalInput")
    acc_d = nc.dram_tensor("acc", [P, N_ACC], F32, kind="ExternalOutput")

    w_t = w_d[:, :].rearrange("(n p) m -> n p m", p=P)
    t_t = t_d[:, :].rearrange("(n p) m -> n p m", p=P)

    with TileContext(nc) as tc:
        with (
            tc.tile_pool(name="iow", bufs=5) as iow_pool,
            tc.tile_pool(name="iot", bufs=4) as iot_pool,
            tc.tile_pool(name="cp", bufs=2) as c_pool,
            tc.tile_pool(name="fp", bufs=2) as f_pool,
            tc.tile_pool(name="dp", bufs=2) as d_pool,
            tc.tile_pool(name="sp", bufs=2) as s_pool,
            tc.tile_pool(name="zp", bufs=3) as z_pool,
            tc.tile_pool(name="ao", bufs=2) as a_pool,
            tc.tile_pool(name="fix", bufs=1) as fix_pool,
        ):
            acc = fix_pool.tile([P, N_ACC], F32)
            bias_nh = fix_pool.tile([P, 1], F32)
            nc.vector.memset(acc[:, :], 0.0)
            nc.vector.memset(bias_nh[:, :], -0.5)

            ws = ts = None
            cur_tile = -1
            for si, (ti, off, fd) in enumerate(SEGS):
                if ti != cur_tile:
                    ws = iow_pool.tile([P, FD], BF16, tag="w")
                    ts = iot_pool.tile([P, FD], BF16, tag="t")
                    for o, dfd in TILE_SUBS[ti]:
                        nc.sync.dma_start(
                            ws[:, o : o + dfd], w_t[ti][:, o : o + dfd]
                        )
                        nc.sync.dma_start(
                            ts[:, o : o + dfd], t_t[ti][:, o : o + dfd]
                        )
                    cur_tile = ti
                wv = ws[:, off : off + fd]
                tv = ts[:, off : off + fd]

                c = c_pool.tile([P, FD], BF16, tag="c")
                F = f_pool.tile([P, FD], BF16, tag="F")
                d2 = d_pool.tile([P, FD], BF16, tag="d2")
                sg = s_pool.tile([P, FD], BF16, tag="sg")
                z = z_pool.tile([P, FD], BF16, tag="z")

                nc.vector.tensor_scalar(c[:, :fd], tv, MAGIC, MAGIC, add, sub)
                nc.vector.tensor_scalar(F[:, :fd], wv, MAGIC, MAGIC, add, sub)
                nc.vector.tensor_scalar(d2[:, :fd], wv, 2.0, 0.5, mult, add)
                nc.vector.tensor_tensor(sg[:, :fd], c[:, :fd], F[:, :fd], add)
                nc.vector.tensor_tensor(z[:, :fd], d2[:, :fd], sg[:, :fd], sub)

                ao = a_pool.tile([P, FD], BF16, tag="ao")
                nc.scalar.activation(
                    ao[:, :fd], z[:, :fd], Relu,
                    bias=bias_nh[:, :], scale=1.0,
                    accum_out=acc[:, 2 * si : 2 * si + 1],
                )
                ao2 = a_pool.tile([P, FD], BF16, tag="ao")
                nc.scalar.activation(
                    ao2[:, :fd], z[:, :fd], Relu,
                    bias=bias_nh[:, :], scale=-1.0,
                    accum_out=acc[:, 2 * si + 1 : 2 * si + 2],
                )

            nc.sync.dma_start(acc_d[:, :], acc[:, :])

    nc.compile()
    return nc


def kernel(input, target):
    global LAST_RESULTS
    import ml_dtypes

    bf = ml_dtypes.bfloat16
    x = np.asarray(input, dtype=np.float32)
    t = np.asarray(target, dtype=np.float32)
    assert x.shape == (B, D) and t.shape == (B, D)

    w = (x - np.float32(0.5)).astype(bf)
    tb = t.astype(bf)

    if "nc" not in _CACHE:
        _CACHE["nc"] = build_nc()
    nc = _CACHE["nc"]

    in_maps = []
    for j in range(N_CORES):
        r0, r1 = j * ROWS, (j + 1) * ROWS
        in_maps.append(
            {
                "w": np.ascontiguousarray(w[r0:r1]),
                "t": np.ascontiguousarray(tb[r0:r1]),
            }
        )

    res = run_bass_kernel_spmd(nc, in_maps, core_ids=list(range(N_CORES)))
    LAST_RESULTS = res

    loss_sum = 0.0
    for j in range(N_CORES):
        loss_sum += res.results[j]["acc"].astype(np.float64).sum()
    return np.float32(loss_sum / (float(B) * float(D)))
